# revision 41
# baseline (speedup 1.0000x reference)
"""Trainium2 Bass kernel for nn_ContrastiveLoss (NT-Xent with sampled negatives).

Reference semantics (B=4096, D=512, N=8192, R=4 negatives/row, temp=0.5+1e-8):
    z  = concat(z_i, z_j)                       [N, D]
    zn = z / max(||z||, 1e-8)
    sim = (zn @ zn.T) / temp
    pos[i]  = sim[i, (i+B) % N]
    cols    = neg_idx + (neg_idx >= row)        (skip-diagonal remap)
    neg[i,k] = sim[i, cols[i,k]]
    nll = logsumexp([pos, neg]) - pos ;  loss = mean(nll)

Only 5 sim entries per row are needed, so the [N, N] matrix is never formed.

Design (v5, fp8 DoubleRow + l-major streaming):
- Paired row sharding: core m owns rows [512m, 512m+512) u [B+512m, B+512m+512),
  so each row's positive partner is local; pos-dots and norms computed once.
- Row data staged host-side in fp8e4m3 (|z| <= ~5.5 << 240 max; the 2e-2
  rel-err budget dwarfs fp8 dot noise, which averages out over 512-long dots).
- Own rows staged in the DoubleRow-transposed layout (d = cc*256 + 2q + two ->
  partition q, k-tile two, chunk cc); negative rows fetched with transpose-mode
  dma_gather, whose 16-bit-granularity transpose lands fp8 byte pairs exactly
  in DoubleRow layout.  Gather cost in this machine model is a fixed
  ~3.3ns/row serialized on the Pool engine (~13.7us total) - the kernel's
  spine - so everything else is arranged to hide under it.
- Gather order f = 4*l + k (l-major), chunked as [256,256,256,128,128] rows:
  each chunk delivers ALL four negatives of a contiguous row range, so the
  chunk's logits, nll, and partial-sum complete immediately and only the
  final 128-row chunk trails the last DMA byte.  Host precomputes the
  skip-diagonal remap (index/address arithmetic) in the staged idx tile.
- Dots and sum-of-squares run on the Tensor engine as M=64 subblock matmuls in
  fp8 DoubleRow perf mode (K=256/matmul, 0.5 cycles/row), subblock pairs
  stacked on the 128 PSUM partitions; per-k columns are stride-4 views of the
  chunk's gather tile.  Row l lives at partition l%128 everywhere.
- Diagonals extracted per chunk with one masked TensorTensor (stacked
  identity, bf16 out) + exact tree-adds + one TensorReduce.
- Inverse norms: inv = exp(-0.5*ln(ss) + 0.5*ln(1/temp)) on ACT.  ln/exp share
  activation-table set 6, loaded once manually at kernel start.
- Logits bounded (|cos|/temp <= ~4) so logsumexp skips the max-shift.
Host sums the 8 per-core partials.
"""

import os
import sys

import numpy as np

if "/opt/trn_rl_repo" not in sys.path:
    sys.path.insert(0, "/opt/trn_rl_repo")

B = 4096
D = 512
N = 2 * B
R = 4  # negatives per row
NCORES = 8
RPC = N // NCORES  # rows per core = 1024
HALF = RPC // 2  # 512; partner(l) = (l + HALF) % RPC
P = 128
NCC = 2  # contraction chunks of 256 (DoubleRow: 2 k-tiles of 128)
MSUB = 64  # matmul subblock size
NSEG = RPC // P  # 8 segments: row l -> (partition l%128, segment l//128)
NF = R * RPC  # 4096 gathered rows, f = 4*l + k
# chunk row counts (sum = RPC).  One transpose-gather per chunk; real HW
# wedges (NRT_EXEC_UNIT_UNRECOVERABLE) on transpose gathers above 512
# indices, so chunks are capped at 128 rows (512 gathered rows each).
CH_ROWS = [128] * 8
TEMP = 0.5 + 1e-08
INV_TEMP = float(1.0 / TEMP)
HALF_LN_INV_TEMP = float(0.5 * np.log(INV_TEMP))

_CACHE = {}


def build_nc():
    import concourse.bass as bass
    import concourse.bacc as bacc
    import concourse.mybir as mybir
    from concourse.tile import TileContext

    fp32 = mybir.dt.float32
    bf16 = mybir.dt.bfloat16
    fp8 = mybir.dt.float8e4
    i16 = mybir.dt.int16
    AF = mybir.ActivationFunctionType
    OP = mybir.AluOpType
    AX = mybir.AxisListType
    DR = mybir.MatmulPerfMode.DoubleRow

    nc = bacc.Bacc()
    zh = nc.dram_tensor("zh", [N, D], fp8, kind="ExternalInput")
    # own rows DoubleRow-transposed: zt[q, ((cc*2+two)*RPC + l)] =
    #   z[gl(l), cc*256 + 2q + two]
    zt = nc.dram_tensor("zt", [P, NCC * 2 * RPC], fp8, kind="ExternalInput")
    # final gather columns (host-remapped), int16, order f = 4l + k at
    # position (f%16, f//16), 16-partition pattern replicated to 128.
    idxr = nc.dram_tensor("idxr", [P, NF // 16], i16, kind="ExternalInput")
    # stacked identity mask: eyed[p, n] = (n == p % 64), bf16
    eyed = nc.dram_tensor("eyed", [P, MSUB], bf16, kind="ExternalInput")
    out_partial = nc.dram_tensor("partial", [P, NSEG], fp32, kind="ExternalOutput")
    dbg = os.environ.get("K_DEBUG", "0") == "1"
    if dbg:
        out_logit = nc.dram_tensor(
            "logit_out", [P, NSEG, 1 + R], fp32, kind="ExternalOutput"
        )
        out_ssa = nc.dram_tensor("ssa_out", [P, NSEG], fp32, kind="ExternalOutput")
        out_pos = nc.dram_tensor("pos_out", [P, NSEG // 2], fp32, kind="ExternalOutput")
        out_nll = nc.dram_tensor("nll_out", [P, NSEG], fp32, kind="ExternalOutput")

    with TileContext(nc) as tc:
        with (
            tc.tile_pool(name="big", bufs=1) as big,
            tc.tile_pool(name="scr", bufs=2) as scr,
            tc.tile_pool(name="small", bufs=1) as small,
            tc.tile_pool(name="psA", bufs=3, space="PSUM") as psA,
        ):
            # ---- manual act-table load: set 6 holds ln+exp+square ----
            lset = mybir.InstLoadActFuncSet(
                name=nc.get_next_instruction_name(), ins=[], outs=[]
            )
            lset.act_func_set_id = 6
            nc.scalar.add_instruction(lset)

            # ---- DMA in: indices first (gathers depend on them) ----
            ixr = small.tile([P, NF // 16], i16, tag="ixr")
            nc.sync.dma_start(out=ixr[:], in_=idxr[:])
            eye = small.tile([P, MSUB], bf16, tag="eye")
            nc.sync.dma_start(out=eye[:], in_=eyed[:])
            # own rows: [q, cc, two, l]; split DMA so PE can start early
            aT = big.tile([P, NCC, 2, RPC], fp8, tag="aT")
            zt_v = zt[:].rearrange("p (c t l) -> p c t l", c=NCC, t=2)
            nc.sync.dma_start(out=aT[:, :, :, 0:HALF], in_=zt_v[:, :, :, 0:HALF])
            nc.sync.dma_start(out=aT[:, :, :, HALF:RPC], in_=zt_v[:, :, :, HALF:RPC])

            # ---- chunked transpose gathers (f = 4l + k, l-major chunks) ----
            # chunk [F0, F1): per partition q, byte o = cc*2*CHN + 2*(f-F0) + b
            # holds z[col_f, cc*256 + 2q + b], CHN = F1 - F0.
            gtk = []
            ch_f = [0]
            for rws in CH_ROWS:
                ch_f.append(ch_f[-1] + 4 * rws)
            for ci in range(len(CH_ROWS)):
                F0, F1 = ch_f[ci], ch_f[ci + 1]
                chn = F1 - F0
                g_t = big.tile([P, 2 * NCC * chn], fp8, tag=f"gT{ci}")
                gtk.append(g_t)
                nc.gpsimd.dma_gather(
                    out_ap=g_t[:].rearrange("q (x l) -> q x l", x=4),
                    in_ap=zh[:],
                    idxs_ap=ixr[:, F0 // 16 : F1 // 16],
                    num_idxs=chn,
                    num_idxs_reg=chn,
                    elem_size=D,
                    transpose=True,
                )

            def a_sl(cc, a0, b=None):
                if b is None:
                    return aT[:, cc, :, a0 : a0 + MSUB]
                return aT[:, cc, b, a0 : a0 + MSUB]

            # per-chunk view [q, cc, c=2k+b, l]: byte o = cc*2CHN + 8l + 2k + b
            gvk = [
                g_t[:].rearrange("q (cc l c) -> q cc c l", cc=NCC, c=8)
                for g_t in gtk
            ]

            def g_sl(ci, cc, k, l0, b=None):
                """View of chunk ci for negatives k of own rows [l0, l0+64)
                (chunk-relative): [q, b(2), l(64)] for DoubleRow (b=None), or
                [q, l(64)] at a fixed byte b for plain matmuls."""
                if b is None:
                    return gvk[ci][:, cc, 2 * k : 2 * k + 2, l0 : l0 + MSUB]
                return gvk[ci][:, cc, 2 * k + b, l0 : l0 + MSUB]

            def block(ps, seg, half, lhs_fn, rhs_fn, w_dr=True):
                """One stacked M=64 matmul group into ps[64h:64h+64, seg, :]
                (diag = wanted dots).  fp8 DoubleRow only where legal: dst
                partition 0 (walrus s3d3_mm_valid_dst_partition) AND weights
                in the contiguous aT layout (s3_lw_dual_fp8_restrictions
                rejects the byte-interleaved gather view as weights).
                Otherwise 4 accumulating plain K=128 matmuls over (cc, byte)."""
                outp = ps[64 * half : 64 * (half + 1), seg, :]
                if half == 0 and w_dr:
                    for cc in range(NCC):
                        nc.tensor.matmul(
                            out=outp,
                            lhsT=lhs_fn(cc, None),
                            rhs=rhs_fn(cc, None),
                            start=(cc == 0),
                            stop=(cc == NCC - 1),
                            perf_mode=DR,
                        )
                else:
                    n = 0
                    for cc in range(NCC):
                        for b in range(2):
                            nc.tensor.matmul(
                                out=outp,
                                lhsT=lhs_fn(cc, b),
                                rhs=rhs_fn(cc, b),
                                start=(n == 0),
                                stop=(n == 2 * NCC - 1),
                            )
                            n += 1

            def extract(ps, nseg, tag):
                """Masked diag extract of [128, nseg, 64] psum -> [128, nseg, 1]."""
                mk = scr.tile([P, nseg, MSUB], bf16, tag=f"mk{nseg}_{tag[:1]}")
                nc.vector.tensor_tensor(
                    out=mk[:],
                    in0=ps,
                    in1=eye[:]
                    .rearrange("p (o l) -> p o l", o=1)
                    .to_broadcast([P, nseg, MSUB]),
                    op=OP.mult,
                )
                w = MSUB
                cur = mk
                while w > 8:
                    h = w // 2
                    nxt = scr.tile([P, nseg, h], bf16, tag=f"tr{nseg}_{h}_{tag[:1]}")
                    nc.vector.tensor_tensor(
                        out=nxt[:], in0=cur[:, :, 0:h], in1=cur[:, :, h:w], op=OP.add
                    )
                    cur = nxt
                    w = h
                red = small.tile([P, nseg, 1], fp32, tag=f"red_{tag}")
                nc.vector.tensor_reduce(out=red[:], in_=cur[:], axis=AX.X, op=OP.add)
                return red

            # ---- prologue on PE while gathers run: own ss + pos dots ----
            ps_own = psA.tile([P, 16, MSUB], fp32, tag="ps")
            for s in range(NSEG):
                for h in range(2):
                    a0 = 128 * s + 64 * h
                    fa = lambda cc, b, a0=a0: a_sl(cc, a0, b)
                    block(ps_own, s, h, fa, fa)
            for s in range(NSEG // 2):
                for h in range(2):
                    a0 = 128 * s + 64 * h
                    block(ps_own, NSEG + s, h,
                          lambda cc, b, a0=a0: a_sl(cc, a0, b),
                          lambda cc, b, a0=a0: a_sl(cc, HALF + a0, b))
            own_red = extract(ps_own[:, 0:12, :], 12, "own")
            ssa = own_red[:, 0:NSEG, :]  # [128, 8, 1]
            posd = own_red[:, NSEG:12, :]  # [128, 4, 1]

            # inva = exp(-0.5*ln(ssa) + 0.5*ln(1/temp))  [128, 8]
            bias_t = small.tile([P, 1], fp32, tag="bias_t")
            nc.vector.memset(bias_t[:], HALF_LN_INV_TEMP)
            ln_a = small.tile([P, NSEG], fp32, tag="ln_a")
            nc.scalar.activation(
                out=ln_a[:], in_=ssa[:].rearrange("p s o -> p (s o)"), func=AF.Ln
            )
            inva = small.tile([P, NSEG], fp32, tag="inva")
            nc.scalar.activation(
                out=inva[:], in_=ln_a[:], func=AF.Exp, scale=-0.5, bias=bias_t[:]
            )

            # ---- pos logit (early): posd[s%4] * inva[s] * inva[(s+4)%8] ----
            logit = small.tile([P, NSEG, 1 + R], fp32, tag="logit")
            lp = logit[:, :, 0:1]
            H = NSEG // 2
            pos_f = small.tile([P, NSEG], fp32, tag="pos_f")
            pd = posd[:].rearrange("p s o -> p (s o)")
            nc.vector.tensor_tensor(
                out=pos_f[:, 0:H], in0=pd, in1=inva[:, 0:H], op=OP.mult
            )
            nc.vector.tensor_tensor(
                out=pos_f[:, H:NSEG], in0=pd, in1=inva[:, H:NSEG], op=OP.mult
            )
            swap = small.tile([P, NSEG], fp32, tag="swap")
            nc.vector.tensor_copy(out=swap[:, 0:H], in_=inva[:, H:NSEG])
            nc.vector.tensor_copy(out=swap[:, H:NSEG], in_=inva[:, 0:H])
            nc.vector.tensor_tensor(
                out=lp.rearrange("p s o -> p (s o)"), in0=pos_f[:], in1=swap[:],
                op=OP.mult,
            )

            # ---- per chunk: PE blocks, extract, logits, nll ----
            nll = small.tile([P, NSEG], fp32, tag="nll")
            row0 = 0
            for ci, rws in enumerate(CH_ROWS):
                last = ci == len(CH_ROWS) - 1
                nrb = rws // P  # row-blocks (segments) in this chunk
                s0 = row0 // P  # first global segment
                ns = R * nrb  # per-family psum segs (k-major: k*nrb + rb)
                ps_k = psA.tile([P, 16, MSUB], fp32, tag="ps")
                # ss blocks then dot blocks; seg = fam*ns + k*nrb + rb
                for fam in range(2):
                    for k in range(R):
                        for rb in range(nrb):
                            for h in range(2):
                                l0 = 128 * rb + 64 * h  # chunk-relative
                                a0 = 128 * (s0 + rb) + 64 * h
                                gfn = lambda cc, b, ci=ci, k=k, l0=l0: g_sl(
                                    ci, cc, k, l0, b
                                )
                                lfn = (
                                    gfn
                                    if fam == 0
                                    else (lambda cc, b, a0=a0: a_sl(cc, a0, b))
                                )
                                block(ps_k, fam * ns + k * nrb + rb, h, lfn, gfn,
                                      w_dr=(fam == 1))
                if last:
                    # split extract: ss first so ACT ln/exp overlaps dot extract
                    red_ss = extract(ps_k[:, 0:ns, :], ns, f"s{ci}")
                    red_dot = extract(ps_k[:, ns : 2 * ns, :], ns, f"d{ci}")
                else:
                    red_k = extract(ps_k[:, 0 : 2 * ns, :], 2 * ns, f"c{ci}")
                    red_ss = red_k[:, 0:ns, :]
                    red_dot = red_k[:, ns : 2 * ns, :]
                # invg for this chunk  [128, ns] (k-major)
                ln_g = small.tile([P, ns], fp32, tag=f"ln_g{ci}")
                nc.scalar.activation(
                    out=ln_g[:],
                    in_=red_ss.rearrange("p s o -> p (s o)"),
                    func=AF.Ln,
                )
                invg = small.tile([P, ns], fp32, tag=f"invg{ci}")
                nc.scalar.activation(
                    out=invg[:], in_=ln_g[:], func=AF.Exp, scale=-0.5, bias=bias_t[:]
                )
                # neg logits: logit[:, s0:s0+nrb, k+1] = dot*invg*inva
                for k in range(R):
                    lgk = logit[:, s0 : s0 + nrb, k + 1 : k + 2]
                    nc.vector.tensor_tensor(
                        out=lgk,
                        in0=red_dot[:, k * nrb : (k + 1) * nrb, :],
                        in1=invg[:, k * nrb : (k + 1) * nrb].rearrange(
                            "p (s o) -> p s o", o=1
                        ),
                        op=OP.mult,
                    )
                    nc.vector.tensor_tensor(
                        out=lgk, in0=lgk,
                        in1=inva[:, s0 : s0 + nrb].rearrange("p (s o) -> p s o", o=1),
                        op=OP.mult,
                    )
                # nll slice = ln(sum exp(logit)) - lp
                if ci == 0:
                    ex = small.tile([P, NSEG, 1 + R], fp32, tag="ex")
                sume = small.tile([P, nrb, 1], fp32, tag=f"sume{ci}")
                if last:
                    # fused exp+accumulate per segment (one ACT op per seg)
                    for rb in range(nrb):
                        nc.scalar.activation(
                            out=ex[:, s0 + rb, :], in_=logit[:, s0 + rb, :],
                            func=AF.Exp, accum_out=sume[:, rb, :],
                        )
                else:
                    nc.scalar.activation(
                        out=ex[:, s0 : s0 + nrb, :], in_=logit[:, s0 : s0 + nrb, :],
                        func=AF.Exp,
                    )
                    nc.vector.tensor_reduce(
                        out=sume[:], in_=ex[:, s0 : s0 + nrb, :], axis=AX.X,
                        op=OP.add,
                    )
                lns = small.tile([P, nrb], fp32, tag=f"lns{ci}")
                nc.scalar.activation(
                    out=lns[:], in_=sume[:].rearrange("p s o -> p (s o)"), func=AF.Ln
                )
                nc.vector.tensor_tensor(
                    out=nll[:, s0 : s0 + nrb], in0=lns[:],
                    in1=lp[:, s0 : s0 + nrb, :].rearrange("p s o -> p (s o)"),
                    op=OP.subtract,
                )
                row0 += rws

            # ---- output per-row nll; host sums ----
            nc.sync.dma_start(out=out_partial[:], in_=nll[:])
            if dbg:
                nc.sync.dma_start(out=out_nll[:], in_=nll[:])
                nc.sync.dma_start(out=out_logit[:], in_=logit[:])
                nc.sync.dma_start(
                    out=out_ssa[:], in_=ssa[:].rearrange("p s o -> p (s o)")
                )
                nc.sync.dma_start(
                    out=out_pos[:], in_=posd[:].rearrange("p s o -> p (s o)")
                )

    nc.finalize()
    return nc


def make_in_maps(z_i, z_j, neg_idx):
    import concourse.mybir as mybir

    f8 = mybir.dt.np(mybir.dt.float8e4)
    import ml_dtypes

    bf = ml_dtypes.bfloat16
    z = np.concatenate([z_i, z_j], axis=0).astype(np.float32)
    zh = np.ascontiguousarray(z.astype(f8))
    neg_idx = np.asarray(neg_idx, dtype=np.int64)
    eye = np.zeros((P, MSUB), dtype=np.float32)
    eye[np.arange(P), np.arange(P) % MSUB] = 1.0
    eye = np.ascontiguousarray(eye.astype(bf))

    l = np.arange(RPC)
    in_maps = []
    for m in range(NCORES):
        gl = np.where(l < HALF, HALF * m + l, B + HALF * m + (l - HALF))
        zo = zh[gl]  # [1024, 512] fp8
        # zt[q, cc, two, l] = zo[l, cc*256 + 2q + two]
        zt = np.ascontiguousarray(
            zo.reshape(RPC, NCC, P, 2).transpose(2, 1, 3, 0).reshape(P, -1)
        )
        # final gather columns with skip-diagonal remap, order f = 4l + k
        cols_f = np.empty(NF, dtype=np.int16)
        for k in range(R):
            nk = neg_idx[gl, k]
            cols_f[4 * l + k] = (nk + (nk >= gl)).astype(np.int16)
        fa = np.arange(NF)
        pat = np.zeros((16, NF // 16), dtype=np.int16)
        pat[fa % 16, fa // 16] = cols_f
        idxr = np.ascontiguousarray(np.tile(pat, (P // 16, 1)))
        in_maps.append({"zh": zh, "zt": zt, "idxr": idxr, "eyed": eye})
    return in_maps


def kernel(z_i, z_j, neg_idx, _bench=None):
    from concourse.bass_utils import run_bass_kernel_spmd

    if "nc" not in _CACHE:
        _CACHE["nc"] = build_nc()
    nc = _CACHE["nc"]
    in_maps = make_in_maps(z_i, z_j, neg_idx)
    core_ids = list(range(NCORES))
    kw = dict(_bench or {})
    r = run_bass_kernel_spmd(nc, in_maps, core_ids, **kw)
    if _bench is not None:
        _CACHE["last_results"] = r
    total = np.sum(
        [np.asarray(r.results[m]["partial"], dtype=np.float64).sum() for m in range(NCORES)]
    )
    return np.float32(total / N)
